# revision 7
# baseline (speedup 1.0000x reference)
"""GCN encoder (dense+relu -> GCNConv -> {mu, logstd} GCNConv) on 8 Trainium2
NeuronCores.

Strategy (v2):
  - Nodes sharded across 8 cores (12500 rows each, padded to 12544 = 98*128).
  - Edges partitioned by destination shard, sorted by (dest window of 128,
    source piece, source), laid out as 128-edge tiles.
  - S (one-hot scatter matrices with edge weights) built ON DEVICE from
    compact per-tile (dstoff, ew) vectors via DVE tensor_scalar
    (iota == dstoff) * ew  -- no S DMA traffic.
  - Transposed aggregation: psT[feat, dest] += msg_chunk^T @ S per tile,
    so the 256x256 weight transform consumes psT chunks directly as lhsT
    (no PE transposes), in f16.
  - deg/dinv/sqrt(deg) computed on host; passed as small tiles.
  - 4 AllGather pieces (windows [24,24,24,26]) pipelined with producers;
    chunk == piece for the gathers.
  - Gather index padding: per (group, chunk) call the trailing pad slots
    (common across cores) are -1 -> skipped by SWDGE/DMA.
"""
import sys

sys.path.insert(0, "/opt/trn_rl_repo")

import numpy as np

import concourse.bacc as bacc
import concourse.bass as bass
import concourse.mybir as mybir
from concourse import tile
from concourse.bass_utils import run_bass_kernel_spmd
from concourse.masks import make_identity

F32 = mybir.dt.float32
F16 = mybir.dt.float16
I16 = mybir.dt.int16


class Cfg:
    def __init__(self, N=100000, NC=8, DIN=256, DMID=256, DOUT=256,
                 group_w=2, stage=3, skip_neg=True):
        assert N % NC == 0
        self.N, self.NC = N, NC
        self.DIN, self.DMID, self.DOUT = DIN, DMID, DOUT
        self.SH = N // NC                       # real rows per shard
        self.SHP = -(-self.SH // 256) * 256     # padded rows per shard (12544)
        self.W = 128                            # dest window size
        self.NW = self.SHP // self.W            # windows per shard (98)
        # AllGather pieces, in windows (sum == NW); last absorbs remainder
        self.PIECE_W = [24, 24, 24, self.NW - 72]
        self.NPIECE = len(self.PIECE_W)
        self.HSH = [pw * self.W for pw in self.PIECE_W]   # rows per piece
        self.PLB = np.concatenate([[0], np.cumsum(self.HSH)])  # local bounds
        self.CH = [NC * h for h in self.HSH]    # gather chunk rows (global)
        assert all(c <= 32767 for c in self.CH)
        self.NCHUNK = self.NPIECE
        self.GROUP_W = group_w
        self.SKIP_NEG = skip_neg
        self.stage = stage  # 1=dense only, 2=+conv1, 3=full


def _preprocess(x, edge_index, edge_attr, cfg: Cfg):
    """Build per-core input arrays and the (core-independent) tile schedule."""
    N, NC, SH, SHP = cfg.N, cfg.NC, cfg.SH, cfg.SHP
    NCHUNK, W, NW = cfg.NCHUNK, cfg.W, cfg.NW

    src = np.asarray(edge_index[0], dtype=np.int64)
    dst = np.asarray(edge_index[1], dtype=np.int64)
    ew = np.asarray(edge_attr, dtype=np.float32)
    # self-loops (weight 1) are handled separately on-device via an
    # identity matmul of the core's own rows; they only enter the degree.
    core = dst // SH
    ldst = dst - core * SH
    # source -> (chunk, within-chunk row); chunk == AllGather piece
    s_shard = src // SH
    s_loc = src - s_shard * SH
    s_piece = np.searchsorted(cfg.PLB, s_loc, side="right") - 1
    cidx = (s_shard * np.asarray(cfg.HSH)[s_piece]
            + (s_loc - cfg.PLB[s_piece])).astype(np.int16)
    chunk = s_piece
    win = ldst // W
    dstoff = (ldst - win * W).astype(np.int64)

    # cell = (core, win, chunk); sort edges by (cell, src) for locality
    cell = (core * NW + win) * NCHUNK + chunk
    order = np.argsort(cell * N + src, kind="stable")
    cell_s = cell[order]
    n_cells = NC * NW * NCHUNK
    counts = np.bincount(cell, minlength=n_cells).reshape(NC, NW, NCHUNK)
    cmax = counts.max(axis=0)                   # [NW, NCHUNK]

    # common schedule: tiles per (win, chunk) = max over cores
    T_wc = -(-cmax // 128)                      # [NW, NCHUNK]
    tile_base = np.zeros((NW, NCHUNK), np.int64)
    tile_base.ravel()[1:] = np.cumsum(T_wc.ravel())[:-1]
    TOT = int(T_wc.sum())

    # per-edge slot row within its core's S-layout arrays
    starts = np.zeros(n_cells + 1, np.int64)
    starts[1:] = np.cumsum(counts.ravel())
    rank_s = np.arange(len(cell_s), dtype=np.int64) - starts[cell_s]
    w_s = (cell_s // NCHUNK) % NW
    c_s = cell_s % NCHUNK
    core_s = cell_s // (NW * NCHUNK)
    erow_s = tile_base[w_s, c_s] * 128 + rank_s

    # compact S representation: per S-layout slot, (dstoff, ew)
    DSTO = np.full((NC, 128, TOT), 1000.0, np.float32)
    EWV = np.zeros((NC, 128, TOT), np.float32)
    DSTO[core_s, erow_s % 128, erow_s // 128] = dstoff[order].astype(np.float32)
    EWV[core_s, erow_s % 128, erow_s // 128] = ew[order].astype(np.float32)
    IDXRAW = np.zeros((NC, TOT * 128), np.int16)
    IDXRAW[core_s, erow_s] = cidx[order]

    # groups of consecutive windows; per-group msg order = (chunk, win, tile)
    # within each (group, chunk) the window with the larger skippable tail
    # goes last so its trailing pad slots can be -1 (skipped by the DMA).
    groups = [list(range(g, min(g + cfg.GROUP_W, NW)))
              for g in range(0, NW, cfg.GROUP_W)]
    perm = []            # msg position -> schedule tile index
    sched = []           # per group: dict with layout info
    for ws in groups:
        base_msg = len(perm)
        c_off = []       # msg offset (within group) of each chunk's block
        c_ord = []       # per chunk: window processing order
        c_reg = []       # per chunk: num_idxs_reg (valid idx count)
        off = 0
        for c in range(NCHUNK):
            tail = {w: int(T_wc[w, c]) * 128 - int(cmax[w, c]) for w in ws}
            order_w = sorted(ws, key=lambda w: tail[w])  # big tail last
            c_off.append(off)
            c_ord.append(order_w)
            tgc = 0
            for w in order_w:
                for t in range(T_wc[w, c]):
                    perm.append(tile_base[w, c] + t)
                tgc += int(T_wc[w, c])
            off += tgc
            wl = order_w[-1]
            n_skip = (tail[wl] if T_wc[wl, c] > 0 else 0) if cfg.SKIP_NEG else 0
            c_reg.append(tgc * 128 - n_skip)
            # mark the common trailing pad slots of the last window as -1
            if cfg.SKIP_NEG and T_wc[wl, c] > 0 and n_skip > 0:
                s0 = tile_base[wl, c] * 128 + int(cmax[wl, c])
                s1 = (tile_base[wl, c] + int(T_wc[wl, c])) * 128
                IDXRAW[:, s0:s1] = -1
        sched.append(dict(ws=ws, base_msg=base_msg, c_off=c_off,
                          c_ord=c_ord, c_reg=c_reg,
                          tg=off, s_base=int(tile_base[ws[0], 0])))
    perm = np.asarray(perm, dtype=np.int64)
    assert len(perm) == TOT
    TGMAX = max(g["tg"] for g in sched)

    rows_perm = (perm[:, None] * 128 + np.arange(128)).reshape(-1)
    idxg = IDXRAW[:, rows_perm]                       # [NC, TOT*128]
    idxg = idxg.reshape(NC, TOT * 8, 16).transpose(0, 2, 1)   # [NC,16,TOT*8]
    IDXG = np.tile(idxg, (1, 8, 1))                   # [NC, 128, TOT*8]

    # host-side degree -> dinv tiles (self-loop weight 1 included)
    deg = np.ones(NC * SHP, np.float32)
    np.add.at(deg, core * SHP + ldst, ew)
    deg = deg.reshape(NC, SHP)
    dinv = 1.0 / np.sqrt(deg)
    dinv2 = 1.0 / deg
    sqd = np.sqrt(deg)
    DINV = dinv.reshape(NC, NW, 128).transpose(0, 2, 1).copy()    # [NC,128,NW]
    DINV2 = dinv2.reshape(NC, NW, 128).transpose(0, 2, 1).copy()
    SQDROW = sqd.reshape(NC, 1, SHP).astype(np.float16)

    # x: pad, shard, transpose to [DIN, SHP], f16
    xsT = np.zeros((NC, cfg.DIN, SHP), np.float16)
    xs = np.asarray(x, np.float32).reshape(NC, SH, cfg.DIN)
    xsT[:, :, :SH] = xs.transpose(0, 2, 1).astype(np.float16)

    meta = dict(T_wc=T_wc, TOT=TOT, TGMAX=TGMAX, groups=groups, sched=sched,
                tile_base=tile_base)
    data = dict(xsT=xsT, dsto=DSTO, ewv=EWV, gidx=IDXG,
                dinv=DINV.astype(np.float32), dinv2=DINV2.astype(np.float32),
                sqdrow=SQDROW)
    return meta, data


def _build_program(cfg: Cfg, meta):
    NC, SHP, NCHUNK, W, NW = cfg.NC, cfg.SHP, cfg.NCHUNK, cfg.W, cfg.NW
    DIN, DMID, DOUT = cfg.DIN, cfg.DMID, cfg.DOUT
    T_wc, TOT, TGMAX, sched = (meta["T_wc"], meta["TOT"], meta["TGMAX"],
                               meta["sched"])
    tile_base = meta["tile_base"]
    NRT = SHP // 128                  # row tiles per shard
    GW = cfg.GROUP_W

    nc = bacc.Bacc("TRN2", target_bir_lowering=False, debug=False,
                   num_devices=NC, num_swdge_queues=4)

    xsT = nc.dram_tensor("xsT", [DIN, SHP], F16, kind="ExternalInput")
    dsto = nc.dram_tensor("dsto", [128, TOT], F32, kind="ExternalInput")
    ewv = nc.dram_tensor("ewv", [128, TOT], F32, kind="ExternalInput")
    gidx = nc.dram_tensor("gidx", [128, TOT * 8], I16, kind="ExternalInput")
    dinv_d = nc.dram_tensor("dinv", [128, NRT], F32, kind="ExternalInput")
    dinv2_d = nc.dram_tensor("dinv2", [128, NRT], F32, kind="ExternalInput")
    sqdrow_d = nc.dram_tensor("sqdrow", [1, SHP], F16, kind="ExternalInput")
    wd = nc.dram_tensor("wd", [DIN, DMID], F16, kind="ExternalInput")
    bd = nc.dram_tensor("bd", [1, DMID], F16, kind="ExternalInput")
    we = nc.dram_tensor("we", [DMID, DMID], F16, kind="ExternalInput")
    be = nc.dram_tensor("be", [1, DMID], F16, kind="ExternalInput")
    wc = nc.dram_tensor("wc", [DMID, DOUT], F16, kind="ExternalInput")
    bc = nc.dram_tensor("bc", [1, DOUT], F16, kind="ExternalInput")
    out = nc.dram_tensor("out", [SHP, DOUT], F16, kind="ExternalOutput")

    # per-piece shard buffers and allgathered tables
    u0s = [nc.dram_tensor(f"u0s{p}", [cfg.HSH[p], DMID], F16)
           for p in range(cfg.NPIECE)]
    u0f = [nc.dram_tensor(f"u0f{p}", [NC * cfg.HSH[p], DMID], F16,
                          addr_space="Shared") for p in range(cfg.NPIECE)]
    u1s = [nc.dram_tensor(f"u1s{p}", [cfg.HSH[p], DMID], F16)
           for p in range(cfg.NPIECE)]
    u1f = [nc.dram_tensor(f"u1f{p}", [NC * cfg.HSH[p], DMID], F16,
                          addr_space="Shared") for p in range(cfg.NPIECE)]

    rg = [list(range(NC))]
    PLB = cfg.PLB

    def piece_of(w):
        return int(np.searchsorted(PLB, w * 128, side="right") - 1)

    def shard_rows(dram_list, w, nw=1):
        """DRAM slice for nw*128 rows starting at w*128 in piece buffers."""
        p = piece_of(w)
        off = w * 128 - int(PLB[p])
        assert w * 128 + nw * 128 <= int(PLB[p + 1]), "crosses piece boundary"
        return dram_list[p][off:off + nw * 128, :]

    with tile.TileContext(nc) as tc:
        with (
            tc.tile_pool(name="const", bufs=1) as cpool,
            tc.tile_pool(name="work", bufs=4) as wpool,
            tc.tile_pool(name="spmm", bufs=4) as gpool,
            tc.tile_pool(name="psum", bufs=2, space="PSUM") as ppool,
        ):
            # ---- constants ----
            ident = cpool.tile([128, 128], F32, tag="ident")
            make_identity(nc, ident[:])
            ident_u = cpool.tile([128, 128], F16, tag="ident_u")
            nc.vector.tensor_copy(out=ident_u[:], in_=ident[:])
            ones1 = cpool.tile([1, 128], F16, tag="ones1")
            nc.vector.memset(ones1[:], 1.0)
            iota_i = cpool.tile([128, 128], I16, tag="iota_i")
            nc.gpsimd.iota(iota_i[:], pattern=[[1, 128]], base=0,
                           channel_multiplier=0)
            iota_f = cpool.tile([128, 128], F16, tag="iota_f")
            nc.vector.tensor_copy(out=iota_f[:], in_=iota_i[:])
            wd_t = [cpool.tile([128, DMID], F16, tag=f"wd{k}", name=f"wd{k}")
                    for k in range(2)]
            we_t = [cpool.tile([128, DMID], F16, tag=f"we{k}", name=f"we{k}")
                    for k in range(2)]
            wc_t = [cpool.tile([128, DOUT], F16, tag=f"wc{k}", name=f"wc{k}")
                    for k in range(2)]
            for k in range(2):
                nc.sync.dma_start(out=wd_t[k][:], in_=wd[k * 128:(k + 1) * 128, :])
                nc.sync.dma_start(out=we_t[k][:], in_=we[k * 128:(k + 1) * 128, :])
                nc.sync.dma_start(out=wc_t[k][:], in_=wc[k * 128:(k + 1) * 128, :])
            bd_t = cpool.tile([1, DMID], F16, tag="bd")
            be_t = cpool.tile([1, DMID], F16, tag="be")
            bc_t = cpool.tile([1, DOUT], F16, tag="bc")
            nc.gpsimd.dma_start(out=bd_t[:], in_=bd[:])
            nc.gpsimd.dma_start(out=be_t[:], in_=be[:])
            nc.gpsimd.dma_start(out=bc_t[:], in_=bc[:])
            dinv = cpool.tile([128, NRT], F32, tag="dinv")
            nc.sync.dma_start(out=dinv[:], in_=dinv_d[:])
            dinv2 = cpool.tile([128, NRT], F32, tag="dinv2")
            nc.sync.dma_start(out=dinv2[:], in_=dinv2_d[:])
            sqdrow = cpool.tile([1, SHP], F16, tag="sqdrow")
            nc.sync.dma_start(out=sqdrow[:], in_=sqdrow_d[:])
            dsto_t = cpool.tile([128, TOT], F32, tag="dsto")
            nc.sync.dma_start(out=dsto_t[:], in_=dsto[:])
            ewv_t = cpool.tile([128, TOT], F32, tag="ewv")
            nc.sync.dma_start(out=ewv_t[:], in_=ewv[:])

            # memset the 4 physical msg buffers once (finite values for
            # skipped trailing gather slots; 0 * finite == 0 in the matmuls)
            for _ in range(4):
                mz = gpool.tile([128, TGMAX * DMID], F16, tag="msg")
                nc.vector.memset(mz[:], 0.0)

            # ---- dense layer: u0 = relu(x @ wd + bd) * dinv ----
            # xT group load: [128, 2 (k-chunk), GW*128] from xsT
            for gi in range(NRT // GW):
                w0 = gi * GW
                xT = wpool.tile([128, 2 * GW * 128], F16, tag="xT")
                nc.sync.dma_start(
                    out=xT[:].rearrange("p (k n) -> p k n", k=2),
                    in_=xsT[:, w0 * 128:(w0 + GW) * 128]
                        .rearrange("(k p) n -> p k n", p=128))
                u0st = wpool.tile([128, GW * DMID], F16, tag="ustage")
                for j in range(GW):
                    w = w0 + j
                    pu = ppool.tile([128, DMID], F32, tag="psu", bufs=3)
                    nc.tensor.matmul(out=pu[:],
                                     lhsT=xT[:, j * 128:(j + 1) * 128],
                                     rhs=wd_t[0][:], start=True, stop=False)
                    nc.tensor.matmul(out=pu[:],
                                     lhsT=xT[:, (GW + j) * 128:(GW + j + 1) * 128],
                                     rhs=wd_t[1][:], start=False, stop=False)
                    nc.tensor.matmul(out=pu[:], lhsT=ones1[:], rhs=bd_t[:],
                                     start=False, stop=True)
                    nc.scalar.activation(out=u0st[:, j * DMID:(j + 1) * DMID],
                                         in_=pu[:],
                                         func=mybir.ActivationFunctionType.Relu,
                                         scale=dinv[:, w:w + 1])
                nc.scalar.dma_start(
                    out=shard_rows(u0s, w0, GW)
                        .rearrange("(t p) d -> p t d", p=128),
                    in_=u0st[:].rearrange("p (t d) -> p t d", d=DMID))
                if cfg.stage >= 2:
                    wlast = w0 + GW - 1
                    if (wlast + 1) * 128 in PLB[1:]:
                        p = piece_of(wlast)
                        nc.gpsimd.collective_compute(
                            "AllGather", mybir.AluOpType.bypass,
                            replica_groups=rg,
                            ins=[u0s[p][:]], outs=[u0f[p][:]])

            if cfg.stage == 1:
                for gi in range(NRT // GW):
                    w0 = gi * GW
                    u0r = wpool.tile([128, GW * DMID], F16, tag="u0r")
                    nc.sync.dma_start(
                        out=u0r[:].rearrange("p (t d) -> p t d", d=DMID),
                        in_=shard_rows(u0s, w0, GW)
                            .rearrange("(t p) d -> p t d", p=128))
                    nc.scalar.dma_start(
                        out=out[w0 * 128:(w0 + GW) * 128, :]
                            .rearrange("(t p) d -> p t d", p=128),
                        in_=u0r[:].rearrange("p (t d) -> p t d", d=DMID))

            def conv(u_full, u_self, w_tiles, b_tile, scale_tile,
                     out_writer, ag_after=False):
                for gi_g, g in enumerate(sched):
                    ws, tg = g["ws"], g["tg"]
                    base_msg, c_off = g["base_msg"], g["c_off"]
                    c_ord, c_reg = g["c_ord"], g["c_reg"]
                    s_base = g["s_base"]
                    w0 = ws[0]
                    msg = gpool.tile([128, TGMAX * DMID], F16, tag="msg")
                    sst = gpool.tile([128, tg * W], F16, tag="sst")
                    gix = gpool.tile([128, tg * 8], I16, tag="gix")
                    nc.sync.dma_start(
                        out=gix[:],
                        in_=gidx[:, base_msg * 8:(base_msg + tg) * 8])
                    for c in range(NCHUNK):
                        tgc = sum(int(T_wc[w, c]) for w in ws)
                        if tgc == 0:
                            continue
                        mo = c_off[c]
                        nc.gpsimd.dma_gather(
                            msg[:, mo * DMID:(mo + tgc) * DMID]
                                .rearrange("p (t d) -> p t d", d=DMID),
                            u_full[c][:],
                            gix[:, mo * 8:(mo + tgc) * 8],
                            num_idxs=tgc * 128,
                            num_idxs_reg=c_reg[c],
                            elem_size=DMID,
                            single_packet=False,
                            queue_num=(gi_g * NCHUNK + c) % 4,
                        )
                    # build S tiles on device: (iota == dstoff) * ew
                    for t in range(tg):
                        nc.vector.tensor_scalar(
                            out=sst[:, t * W:(t + 1) * W],
                            in0=iota_f[:],
                            scalar1=dsto_t[:, s_base + t:s_base + t + 1],
                            scalar2=ewv_t[:, s_base + t:s_base + t + 1],
                            op0=mybir.AluOpType.is_equal,
                            op1=mybir.AluOpType.mult)
                    # batched self-rows load for the group
                    ust = wpool.tile([128, GW * DMID], F16, tag="ust")
                    nc.scalar.dma_start(
                        out=ust[:].rearrange("p (t d) -> p t d", d=DMID),
                        in_=shard_rows(u_self, w0, GW)
                            .rearrange("(t p) d -> p t d", p=128))
                    ustage = wpool.tile([128, GW * DOUT], F16, tag="ustage")
                    for w in ws:
                        j = w - w0
                        psT = ppool.tile([128, DMID], F32, tag="pst")
                        nmm_h = int(T_wc[w, :].sum()) + 1
                        for h in range(2):
                            k = 0
                            nc.tensor.matmul(
                                out=psT[:, h * 128:(h + 1) * 128],
                                lhsT=ust[:, j * DMID + h * 128:
                                         j * DMID + (h + 1) * 128],
                                rhs=ident_u[:],
                                start=True, stop=(k == nmm_h - 1))
                            k += 1
                            for c in range(NCHUNK):
                                pos = c_ord[c].index(w)
                                mo = c_off[c] + sum(int(T_wc[w2, c])
                                                    for w2 in c_ord[c][:pos])
                                so = int(tile_base[w, c]) - s_base
                                for t in range(int(T_wc[w, c])):
                                    nc.tensor.matmul(
                                        out=psT[:, h * 128:(h + 1) * 128],
                                        lhsT=msg[:, (mo + t) * DMID + h * 128:
                                                 (mo + t) * DMID + (h + 1) * 128],
                                        rhs=sst[:, (so + t) * W:(so + t + 1) * W],
                                        start=False, stop=(k == nmm_h - 1))
                                    k += 1
                        s1T = wpool.tile([128, DMID], F16, tag="s1T")
                        for h in range(2):
                            nc.vector.tensor_copy(
                                out=s1T[:, h * 128:(h + 1) * 128],
                                in_=psT[:, h * 128:(h + 1) * 128])
                        pu = ppool.tile([128, DMID], F32, tag="psu", bufs=3)
                        nc.tensor.matmul(out=pu[:], lhsT=s1T[:, :128],
                                         rhs=w_tiles[0][:],
                                         start=True, stop=False)
                        nc.tensor.matmul(out=pu[:], lhsT=s1T[:, 128:],
                                         rhs=w_tiles[1][:],
                                         start=False, stop=False)
                        nc.tensor.matmul(out=pu[:],
                                         lhsT=sqdrow[:, w * 128:(w + 1) * 128],
                                         rhs=b_tile[:],
                                         start=False, stop=True)
                        nc.scalar.activation(
                            out=ustage[:, j * DOUT:(j + 1) * DOUT], in_=pu[:],
                            func=mybir.ActivationFunctionType.Copy,
                            scale=scale_tile[:, w:w + 1])
                    wlast = ws[-1]
                    nc.scalar.dma_start(
                        out=out_writer(w0),
                        in_=ustage[:].rearrange("p (t d) -> p t d", d=DOUT))
                    if ag_after and (wlast + 1) * 128 in PLB[1:]:
                        p = piece_of(wlast)
                        nc.gpsimd.collective_compute(
                            "AllGather", mybir.AluOpType.bypass,
                            replica_groups=rg,
                            ins=[u1s[p][:]], outs=[u1f[p][:]])

            # conv1: u1 = dinv^2*(A@u0)@we + dinv*be   (u1 pre-scaled by dinv)
            if cfg.stage in (2, 3):
                conv(u0f, u0s, we_t, be_t, dinv2,
                     (lambda w0: shard_rows(u1s, w0, GW)
                      .rearrange("(t p) d -> p t d", p=128)) if cfg.stage == 3
                     else (lambda w0: out[w0 * 128:(w0 + GW) * 128, :]
                           .rearrange("(t p) d -> p t d", p=128)),
                     ag_after=(cfg.stage == 3))
            if cfg.stage >= 3:
                # conv2: out = dinv*(A@u1)@wc + bc
                conv(u1f, u1s, wc_t, bc_t, dinv,
                     lambda w0: out[w0 * 128:(w0 + GW) * 128, :]
                     .rearrange("(t p) d -> p t d", p=128))

    nc.compile()
    return nc


def _run(inputs, cfg: Cfg, trace=False):
    x = inputs["x"]
    meta, data = _preprocess(x, inputs["edge_index"], inputs["edge_attr"], cfg)
    nc = _build_program(cfg, meta)

    wcat = np.concatenate([np.asarray(inputs["w_mu"], np.float32),
                           np.asarray(inputs["w_logstd"], np.float32)], axis=1)
    bcat = np.concatenate([np.asarray(inputs["b_mu"], np.float32),
                           np.asarray(inputs["b_logstd"], np.float32)])
    shared = dict(
        wd=np.asarray(inputs["w_dense"], np.float16),
        bd=np.asarray(inputs["b_dense"], np.float16).reshape(1, -1),
        we=np.asarray(inputs["w_enc"], np.float16),
        be=np.asarray(inputs["b_enc"], np.float16).reshape(1, -1),
        wc=wcat.astype(np.float16), bc=bcat.astype(np.float16).reshape(1, -1),
    )
    in_maps = []
    for c in range(cfg.NC):
        m = dict(shared)
        m["xsT"] = np.ascontiguousarray(data["xsT"][c])
        m["dsto"] = np.ascontiguousarray(data["dsto"][c])
        m["ewv"] = np.ascontiguousarray(data["ewv"][c])
        m["gidx"] = np.ascontiguousarray(data["gidx"][c])
        m["dinv"] = np.ascontiguousarray(data["dinv"][c])
        m["dinv2"] = np.ascontiguousarray(data["dinv2"][c])
        m["sqdrow"] = np.ascontiguousarray(data["sqdrow"][c])
        in_maps.append(m)

    res = run_bass_kernel_spmd(nc, in_maps, list(range(cfg.NC)), trace=trace)
    SH = cfg.SH
    halves = cfg.DOUT // 2
    mu = np.concatenate([res.results[c]["out"][:SH, :halves]
                         for c in range(cfg.NC)], axis=0)
    ls = np.concatenate([res.results[c]["out"][:SH, halves:]
                         for c in range(cfg.NC)], axis=0)
    return (mu.astype(np.float32), ls.astype(np.float32)), res


def kernel(**inputs):
    cfg = Cfg()
    (mu, ls), _ = _run(inputs, cfg, trace=False)
    return mu, ls
